# revision 17
# baseline (speedup 1.0000x reference)
"""Distributed Trainium2 kernel for the AEN (attentive episodic network) problem.

Reference computation (shapes):
    support_vs = support @ Wv.T + bv                    [8192, 512]
    q_proto    = queries @ Wv.T + bv                    [8192, 512]
    support_ks = LN(support @ Wk.T + bk)                [8192, 512]
    queries_qs = LN(queries @ Wq.T + bq)                [8192, 512]
    scores     = queries_qs @ support_ks.T / sqrt(512)  [8192, 8192]
    affinity   = softmax(scores, axis=1)
    class_proto= affinity @ support_vs                  [8192, 512]
    returns (q_proto, class_proto)

Sharding: queries split 1024/core across the 8 cores; the support set and
weights are replicated — every core computes the full support K/V projections
locally (collectives on this runtime have a ~2.5 ms latency floor, measured,
so +2x redundant projection FLOPs beats any AllGather by ~5x).

On-chip layouts: activations arrive feature-major ([d, tok] 128x128 blocks,
host pre-packed) so projections emit token-major tiles directly; normalized
K / queries are PE-transposed per 128x128 block into feature-major for the
scores matmul (LN gamma/beta fold into the post-transpose copy as per-
partition scale/bias).  exp(scores.T) tiles [s, q] then serve directly as
lhsT for both attention@V (token-major out, no P transposes) and the softmax
denominator (rhs = ones column).  All matmuls bf16 with f32 PSUM.
"""

import os

import ml_dtypes
import numpy as np

D = 1024  # model dim
O = 512  # out dim
NCORES = 8
NL = 1024  # query rows per core
NS = 8192  # support rows (replicated)
NMT = NL // 128  # 8 query token tiles per core
NST = NS // 128  # 64 support token tiles
NDT = D // 128  # 8 contraction tiles
NOT = O // 128  # 4 outdim tiles
SCALE = 1.0 / float(np.sqrt(np.float32(O)))
LN_EPS = 1e-5
BF16 = ml_dtypes.bfloat16

_CACHE: dict = {}

LAST_RESULTS = None


def _build_graph(reps=1, main_blocks=None):
    import concourse.bass as bass  # noqa: F401
    import concourse.tile as tile
    from concourse import bacc, mybir
    from concourse.masks import make_identity

    f32 = mybir.dt.float32
    bf16 = mybir.dt.bfloat16
    Alu = mybir.AluOpType
    Act = mybir.ActivationFunctionType

    n_st = NST if main_blocks is None else main_blocks * NMT

    nc = bacc.Bacc(
        "TRN2", target_bir_lowering=False, debug=False, num_devices=NCORES
    )

    sTp = nc.dram_tensor("sTp", [NST, NDT, 128, 128], bf16, kind="ExternalInput").ap()
    qTp = nc.dram_tensor("qTp", [NMT, NDT, 128, 128], bf16, kind="ExternalInput").ap()
    w = nc.dram_tensor("w", [D, 3 * O], bf16, kind="ExternalInput").ap()
    bq_b = nc.dram_tensor("bq_b", [128, O], f32, kind="ExternalInput").ap()
    bk_b = nc.dram_tensor("bk_b", [128, O], f32, kind="ExternalInput").ap()
    bv_b = nc.dram_tensor("bv_b", [128, O], f32, kind="ExternalInput").ap()
    g_p = nc.dram_tensor("g_p", [O, 1], f32, kind="ExternalInput").ap()
    be_p = nc.dram_tensor("be_p", [O, 1], f32, kind="ExternalInput").ap()
    out_q = nc.dram_tensor("out_q", [NL, O], f32, kind="ExternalOutput").ap()
    out_c = nc.dram_tensor("out_c", [NL, O], f32, kind="ExternalOutput").ap()

    from contextlib import ExitStack

    with tile.TileContext(nc) as tc:
        with ExitStack() as ctx:
            ent = ctx.enter_context
            consts = ent(tc.tile_pool(name="consts", bufs=1))
            wp = ent(tc.tile_pool(name="wp", bufs=NDT))
            sp = ent(tc.tile_pool(name="sp", bufs=4))
            lnp = ent(tc.tile_pool(name="lnp", bufs=4))
            stp = ent(tc.tile_pool(name="stp", bufs=10))
            yp = ent(tc.tile_pool(name="yp", bufs=3))
            ktp = ent(tc.tile_pool(name="ktp", bufs=NOT))
            qqp = ent(tc.tile_pool(name="qqp", bufs=NOT))
            vlp = ent(tc.tile_pool(name="vlp", bufs=4))
            vsp = ent(tc.tile_pool(name="vsp", bufs=6))
            exl = ent(tc.tile_pool(name="exl", bufs=6))
            ocp = ent(tc.tile_pool(name="ocp", bufs=4))
            psA = ent(tc.tile_pool(name="psA", bufs=4, space="PSUM"))
            psB = ent(tc.tile_pool(name="psB", bufs=3, space="PSUM"))
            psS = ent(tc.tile_pool(name="psS", bufs=1, space="PSUM"))
            dram = ent(tc.tile_pool(name="dram", bufs=1, space="DRAM"))

            ident = consts.tile([128, 128], bf16, name="ident")
            make_identity(nc, ident)
            ones = consts.tile([128, 1], bf16, name="ones")
            nc.vector.memset(ones, 1.0)
            eps_t = consts.tile([128, 1], f32, name="eps_t")
            nc.vector.memset(eps_t, LN_EPS)

            bq_sb = consts.tile([128, O], f32, name="bq_sb")
            nc.sync.dma_start(out=bq_sb, in_=bq_b)
            bk_sb = consts.tile([128, O], f32, name="bk_sb")
            nc.sync.dma_start(out=bk_sb, in_=bk_b)
            bv_sb = consts.tile([128, O], f32, name="bv_sb")
            nc.sync.dma_start(out=bv_sb, in_=bv_b)
            gam = []
            bet = []
            for j in range(NOT):
                g_t = consts.tile([128, 1], f32, name=f"g{j}")
                nc.sync.dma_start(out=g_t, in_=g_p[j * 128 : (j + 1) * 128, :])
                gam.append(g_t)
                b_t = consts.tile([128, 1], f32, name=f"b{j}")
                nc.sync.dma_start(out=b_t, in_=be_p[j * 128 : (j + 1) * 128, :])
                bet.append(b_t)

            wt = []
            for k in range(NDT):
                wtk = wp.tile([128, 3 * O], bf16, name=f"wt{k}", tag="wt")
                nc.sync.dma_start(out=wtk, in_=w[k * 128 : (k + 1) * 128, :])
                wt.append(wtk)

            v_dram = dram.tile([NST * 128 * O], bf16, name="v_dram")

            for _rep in range(reps):
                # K feature-major, resident in SBUF: 4 tiles [128o, 8192s]
                kT = [
                    ktp.tile([128, NS], bf16, name=f"kT{j}", tag="ktp")
                    for j in range(NOT)
                ]
                qqT = [
                    qqp.tile([128, NL], bf16, name=f"qqT{j}", tag="qq")
                    for j in range(NOT)
                ]

                def proj_ln(xt, ps, bias_sb, dstT, m):
                    # bias -> LN stats -> normalize (bf16) -> PE-transpose each
                    # 128 block -> fused gamma*x+beta on the PSUM->SBUF copy
                    pre = lnp.tile([128, O], f32, name="pre", tag="lnp")
                    nc.vector.tensor_add(pre, ps, bias_sb)
                    stats = stp.tile([128, 6], f32, name="stats", tag="stp")
                    nc.vector.bn_stats(stats, pre)
                    mv = stp.tile([128, 2], f32, name="mv", tag="stp")
                    nc.vector.bn_aggr(mv, stats)
                    rstd = stp.tile([128, 1], f32, name="rstd", tag="stp")
                    nc.scalar.activation(
                        rstd, mv[:, 1:2], Act.Sqrt, bias=eps_t, scale=1.0
                    )
                    nc.vector.reciprocal(rstd, rstd)
                    y = yp.tile([128, O], bf16, name="y", tag="yp")
                    nc.vector.tensor_scalar(
                        y, pre, mv[:, 0:1], rstd, Alu.subtract, Alu.mult
                    )
                    for j in range(NOT):
                        pt = psB.tile([128, 128], bf16, name="pt", tag="psB")
                        nc.tensor.transpose(pt, y[:, j * 128 : (j + 1) * 128], ident)
                        nc.scalar.activation(
                            dstT[j][:, m * 128 : (m + 1) * 128],
                            pt,
                            Act.Identity,
                            bias=bet[j],
                            scale=gam[j],
                        )

                # ---- support projections: K (LN, feature-major) + V -> DRAM --
                for ms in range(NST):
                    xt = sp.tile([128, D], bf16, name="xt", tag="sp")
                    nc.sync.dma_start(
                        out=xt,
                        in_=sTp[ms].rearrange("a p b -> p a b"),
                    )
                    ps_k = psA.tile([128, O], f32, name="ps_k", tag="psA")
                    ps_v = psA.tile([128, O], f32, name="ps_v", tag="psA")
                    for k in range(NDT):
                        lhs = xt[:, k * 128 : (k + 1) * 128]
                        nc.tensor.matmul(
                            ps_k, lhs, wt[k][:, O : 2 * O],
                            start=(k == 0), stop=(k == NDT - 1),
                        )
                        nc.tensor.matmul(
                            ps_v, lhs, wt[k][:, 2 * O : 3 * O],
                            start=(k == 0), stop=(k == NDT - 1),
                        )
                    proj_ln(xt, ps_k, bk_sb, kT, ms)
                    v_t = vlp.tile([128, O], bf16, name="v_t", tag="vlp")
                    nc.vector.tensor_add(v_t, ps_v, bv_sb)
                    dst = v_dram[ms * 128 * O : (ms + 1) * 128 * O]
                    nc.sync.dma_start(
                        out=dst.rearrange("(p f) -> p f", p=128), in_=v_t
                    )

                # ---- query projections ----
                for mq in range(NMT):
                    xt = sp.tile([128, D], bf16, name="xt", tag="sp")
                    nc.sync.dma_start(
                        out=xt,
                        in_=qTp[mq].rearrange("a p b -> p a b"),
                    )
                    ps_q = psA.tile([128, O], f32, name="ps_q", tag="psA")
                    ps_pv = psA.tile([128, O], f32, name="ps_pv", tag="psA")
                    for k in range(NDT):
                        lhs = xt[:, k * 128 : (k + 1) * 128]
                        nc.tensor.matmul(
                            ps_q, lhs, wt[k][:, 0:O],
                            start=(k == 0), stop=(k == NDT - 1),
                        )
                        nc.tensor.matmul(
                            ps_pv, lhs, wt[k][:, 2 * O : 3 * O],
                            start=(k == 0), stop=(k == NDT - 1),
                        )
                    proj_ln(xt, ps_q, bq_sb, qqT, mq)
                    qp_sb = lnp.tile([128, O], f32, name="qp_sb", tag="lnp")
                    nc.vector.tensor_add(qp_sb, ps_pv, bv_sb)
                    nc.sync.dma_start(
                        out=out_q[mq * 128 : (mq + 1) * 128, :], in_=qp_sb
                    )

                # ---- attention: 2 query halves of 512, V streamed from DRAM --
                for qh in range(2):
                    sums_ps = psS.tile([128, 4], f32, name="sums_ps", tag="psS")
                    av_ps = [
                        psA.tile([128, O], f32, name=f"av{qi}", tag="psA")
                        for qi in range(4)
                    ]
                    for t in range(n_st):
                        vst = vsp.tile([128, O], bf16, name="vst", tag="vs")
                        src = v_dram[t * 128 * O : (t + 1) * 128 * O]
                        nc.sync.dma_start(
                            out=vst, in_=src.rearrange("(p f) -> p f", p=128)
                        )
                        sc = psB.tile([128, O], f32, name="sc", tag="psB")
                        for j in range(NOT):
                            nc.tensor.matmul(
                                sc,
                                kT[j][:, t * 128 : (t + 1) * 128],
                                qqT[j][:, qh * O : (qh + 1) * O],
                                start=(j == 0),
                                stop=(j == NOT - 1),
                            )
                        ex = exl.tile([128, O], bf16, name="ex", tag="exl")
                        nc.scalar.activation(ex, sc, Act.Exp, scale=SCALE)
                        for qi in range(4):
                            exq = ex[:, qi * 128 : (qi + 1) * 128]
                            nc.tensor.matmul(
                                av_ps[qi], exq, vst,
                                start=(t == 0), stop=(t == n_st - 1),
                            )
                            nc.tensor.matmul(
                                sums_ps[:, qi : qi + 1], exq, ones,
                                start=(t == 0), stop=(t == n_st - 1),
                            )
                    rec = stp.tile([128, 4], f32, name="rec", tag="stp")
                    nc.vector.reciprocal(rec, sums_ps)
                    for qi in range(4):
                        oc = ocp.tile([128, O], f32, name="oc", tag="ocp")
                        nc.vector.tensor_scalar_mul(
                            oc, av_ps[qi], rec[:, qi : qi + 1]
                        )
                        row = (qh * 4 + qi) * 128
                        nc.sync.dma_start(out=out_c[row : row + 128, :], in_=oc)

    nc.compile()
    return nc


def _pack_fm(xT):
    # [D, N] feature-major -> [N/128, D/128, 128, 128] m-major blocks
    n = xT.shape[1]
    return np.ascontiguousarray(
        xT.reshape(NDT, 128, n // 128, 128).transpose(2, 0, 1, 3)
    )


def _prep_inputs(support_set, queries, Wq, bq, Wk, bk, Wv, bv, ln_gamma, ln_beta):
    sT = np.ascontiguousarray(np.asarray(support_set, np.float32).T).astype(BF16)
    qT = np.ascontiguousarray(np.asarray(queries, np.float32).T).astype(BF16)
    sTp = _pack_fm(sT)
    w_cat = np.ascontiguousarray(
        np.concatenate(
            [np.asarray(Wq).T, np.asarray(Wk).T, np.asarray(Wv).T], axis=1
        ).astype(np.float32)
    ).astype(BF16)

    def bc(v):
        return np.ascontiguousarray(
            np.broadcast_to(np.asarray(v, np.float32)[None, :], (128, O))
        )

    shared = {
        "sTp": sTp,
        "w": w_cat,
        "bq_b": bc(bq),
        "bk_b": bc(bk),
        "bv_b": bc(bv),
        "g_p": np.asarray(ln_gamma, np.float32).reshape(O, 1).copy(),
        "be_p": np.asarray(ln_beta, np.float32).reshape(O, 1).copy(),
    }
    in_maps = []
    for i in range(NCORES):
        m = dict(shared)
        m["qTp"] = _pack_fm(np.ascontiguousarray(qT[:, i * NL : (i + 1) * NL]))
        in_maps.append(m)
    return in_maps


def kernel(support_set, queries, Wq, bq, Wk, bk, Wv, bv, ln_gamma, ln_beta):
    global LAST_RESULTS
    from concourse.bass_utils import run_bass_kernel_spmd

    if "nc" not in _CACHE:
        _CACHE["nc"] = _build_graph()
    nc = _CACHE["nc"]

    in_maps = _prep_inputs(
        support_set, queries, Wq, bq, Wk, bk, Wv, bv, ln_gamma, ln_beta
    )
    _CACHE["in_maps"] = in_maps
    res = run_bass_kernel_spmd(
        nc, in_maps, core_ids=list(range(NCORES)), trace=False
    )
    LAST_RESULTS = res
    q_proto = np.concatenate([res.results[i]["out_q"] for i in range(NCORES)], axis=0)
    c_proto = np.concatenate([res.results[i]["out_c"] for i in range(NCORES)], axis=0)
    return (
        np.asarray(q_proto, np.float32),
        np.asarray(c_proto, np.float32),
    )


def _bench_callable(nc):
    """Single-bind jitted callable over 8 cores with device-resident inputs."""
    import jax
    from jax.experimental.shard_map import shard_map
    from jax.sharding import Mesh, NamedSharding, PartitionSpec

    from concourse import bass2jax, mybir

    in_maps = _CACHE["in_maps"]

    partition_name = (
        nc.partition_id_tensor.name if nc.partition_id_tensor else None
    )
    in_names: list[str] = []
    out_names: list[str] = []
    out_avals = []
    zero_outs = []
    for alloc in nc.m.functions[0].allocations:
        if not isinstance(alloc, mybir.MemoryLocationSet):
            continue
        name = alloc.memorylocations[0].name
        if alloc.kind == "ExternalInput":
            if name != partition_name:
                in_names.append(name)
        elif alloc.kind == "ExternalOutput":
            shape = tuple(alloc.tensor_shape)
            dtype = mybir.dt.np(alloc.dtype)
            out_names.append(name)
            out_avals.append(jax.core.ShapedArray(shape, dtype))
            zero_outs.append(np.zeros(shape, dtype))
    n_params = len(in_names)
    in_names_full = list(in_names) + out_names
    if partition_name is not None:
        in_names_full.append(partition_name)

    def _body(*args):
        operands = list(args)
        if partition_name is not None:
            operands.append(bass2jax.partition_id_tensor())
        outs = bass2jax._bass_exec_p.bind(
            *operands,
            out_avals=tuple(out_avals),
            in_names=tuple(in_names_full),
            out_names=tuple(out_names),
            lowering_input_output_aliases=(),
            sim_require_finite=True,
            sim_require_nnan=True,
            nc=nc,
        )
        return tuple(outs)

    devices = jax.devices()[:NCORES]
    mesh = Mesh(np.asarray(devices), ("core",))
    n_outs = len(out_avals)
    in_specs = (PartitionSpec("core"),) * (n_params + n_outs)
    out_specs = (PartitionSpec("core"),) * n_outs
    sharded = jax.jit(
        shard_map(
            _body, mesh=mesh, in_specs=in_specs, out_specs=out_specs,
            check_rep=False,
        )
    )
    per_core = [
        [np.asarray(in_maps[c][name]) for name in in_names] for c in range(NCORES)
    ]
    concat_in = [
        np.concatenate([per_core[c][i] for c in range(NCORES)], axis=0)
        for i in range(n_params)
    ]
    concat_zeros = [
        np.zeros((NCORES * z.shape[0], *z.shape[1:]), z.dtype) for z in zero_outs
    ]
    sh = NamedSharding(mesh, PartitionSpec("core"))
    dev_in = [jax.device_put(a, sh) for a in concat_in]
    dev_zeros = [jax.device_put(a, sh) for a in concat_zeros]
    jax.block_until_ready(dev_in)
    jax.block_until_ready(dev_zeros)

    def run():
        out = sharded(*dev_in, *dev_zeros)
        jax.block_until_ready(out)
        return out

    return run


def benchmark(n_reps=5, timing_reps=8, **graph_kw):
    """Estimate per-execution device time (ns) by unrolling the kernel body
    n_reps times inside one NEFF and differencing against the 1-rep NEFF."""
    import time

    assert "in_maps" in _CACHE, "call kernel() first"
    key1 = ("bnc", 1, tuple(sorted(graph_kw.items())))
    keyN = ("bnc", n_reps, tuple(sorted(graph_kw.items())))
    if key1 not in _CACHE:
        _CACHE[key1] = _build_graph(reps=1, **graph_kw)
    if keyN not in _CACHE:
        _CACHE[keyN] = _build_graph(reps=n_reps, **graph_kw)
    run1 = _bench_callable(_CACHE[key1])
    runN = _bench_callable(_CACHE[keyN])
    run1()
    runN()  # warm compiles

    def best(fn, k):
        ts = []
        for _ in range(k):
            t0 = time.perf_counter()
            fn()
            ts.append(time.perf_counter() - t0)
        return float(np.min(ts))

    t1 = best(run1, timing_reps)
    tN = best(runN, timing_reps)
    per_exec_s = (tN - t1) / (n_reps - 1)
    return per_exec_s * 1e9, t1, tN
